# revision 1
# baseline (speedup 1.0000x reference)
"""Sparse cross-attention kernel for Trainium2 (8 NeuronCores, SPMD).

Problem: nn_CrossAttn (NP=1024 queries, MNP=4096 keys, BS=4, DIM=512,
NH=8 heads, dh=64, TOPK=32 sparse mask shared across heads).

Sharding: core = (batch b, head-group hg).  4 batches x 2 head-groups of 4
heads each.  Each core computes its batch's attention for its 4 heads and
writes a (256 ch, 1024 q) transposed output block; the host reassembles.

Device algorithm (per core), all matmuls bf16 with f32 PSUM accumulation:
  qhT = (W1h/8) @ qT + b1/8          (ch on partitions, queries free)
  khT = W2h @ kT + b2                (ch on partitions, keys free)
  vh  = vT.T @ W3hT + b3             (keys on partitions, ch free)
  per (query-tile qt of 512, head-pair hp, key-chunk kc of 128):
    S^T = Kh @ QhT                   two heads packed in PE array (K=64 row tiles)
    numer = exp(S^T)                 ScalarE, PSUM->SBUF bf16 (scores bounded, no max-sub)
    numerm = numer * maskT[kc]       VectorE, binary mask (exact match w/ reference:
                                      ref adds -1e9 then softmax -> exact zeros)
    OT  += vh[kc].T @ numerm         col-tiled pair, accumulate in PSUM over kc
    den += ones.T @ numerm           M=1 matmuls, accumulate over kc
  out = OT * (1/den broadcast via K=1 matmul)
"""

import numpy as np
import ml_dtypes

import concourse.bass as bass
import concourse.mybir as mybir
import concourse.tile as tile
from concourse.bass_utils import run_bass_kernel_spmd

BF16 = mybir.dt.bfloat16
F32 = mybir.dt.float32
AF = mybir.ActivationFunctionType
ALU = mybir.AluOpType

NH = 8
DIM = 512
NP = 1024
MNP = 4096
BS = 4
DH = 64
N_CORES = 8
HG_CH = 256          # channels per head-group (4 heads x 64)
NKC = MNP // 128     # 32 key chunks
NQT = NP // 512      # 2 query tiles

# options test.py can flip
run_opts = {"trace": False, "trace_kwargs": {}}
_last_results = {}


def _split_multi_waits(nc):
    """This container's walrus encodes only ONE sync-wait per TPB instruction
    (newer Tile emits several).  Split extras onto preceding NOPs."""
    eng_ok = {
        mybir.EngineType.PE,
        mybir.EngineType.Activation,
        mybir.EngineType.DVE,
        mybir.EngineType.Pool,
        mybir.EngineType.SP,
    }
    for fn in nc.m.functions:
        for blk in fn.blocks:
            insts = blk.instructions
            out = []
            changed = False
            for inst in insts:
                si = inst.sync_info
                if (
                    si is not None
                    and si.on_wait
                    and len(si.on_wait) > 1
                    and inst.engine in eng_ok
                ):
                    waits = list(si.on_wait)
                    for j, w in enumerate(waits[:-1]):
                        out.append(
                            mybir.InstNoOp(
                                name=f"{inst.name}-w{j}",
                                engine=inst.engine,
                                ins=[],
                                outs=[],
                                sync_info=mybir.SyncInfo(on_wait=[w], on_update=[]),
                            )
                        )
                    inst.sync_info = mybir.SyncInfo(
                        on_wait=[waits[-1]], on_update=list(si.on_update)
                    )
                    changed = True
                out.append(inst)
            if changed:
                blk.instructions = out


def _build_nc() -> bass.Bass:
    nc = bass.Bass()

    qt_d = nc.dram_tensor("qt", [128, 4, NP], BF16, kind="ExternalInput")
    kt_d = nc.dram_tensor("kt", [128, 4, MNP], BF16, kind="ExternalInput")
    vt_d = nc.dram_tensor("vt", [128, 4, MNP], BF16, kind="ExternalInput")
    w1t_d = nc.dram_tensor("w1t", [128, 4, HG_CH], BF16, kind="ExternalInput")
    w2t_d = nc.dram_tensor("w2t", [128, 4, HG_CH], BF16, kind="ExternalInput")
    w3t_d = nc.dram_tensor("w3t", [128, 4, 260], BF16, kind="ExternalInput")
    b1c_d = nc.dram_tensor("b1c", [128, 2], F32, kind="ExternalInput")
    b2c_d = nc.dram_tensor("b2c", [128, 2], F32, kind="ExternalInput")
    b3r_d = nc.dram_tensor("b3r", [1, 260], BF16, kind="ExternalInput")
    mask_d = nc.dram_tensor("maskt", [128, NKC, NP], BF16, kind="ExternalInput")
    ones_d = nc.dram_tensor("onesc", [128, 1], BF16, kind="ExternalInput")
    ones128_d = nc.dram_tensor("ones128", [1, 128], BF16, kind="ExternalInput")
    ones64_d = nc.dram_tensor("ones64", [1, 64], F32, kind="ExternalInput")
    out_d = nc.dram_tensor("outt", [2, 128, NQT, 512], F32, kind="ExternalOutput")

    with tile.TileContext(nc) as tc:
        with (
            tc.tile_pool(name="const", bufs=1) as const,
            tc.tile_pool(name="big", bufs=1) as big,
            tc.tile_pool(name="kio", bufs=3) as kio,
            tc.tile_pool(name="vio", bufs=3) as vio,
            tc.tile_pool(name="work", bufs=4) as work,
            tc.tile_pool(name="fin", bufs=2) as fin,
            tc.tile_pool(name="ps_s", bufs=2, space="PSUM") as ps_s,
            tc.tile_pool(name="ps_o", bufs=1, space="PSUM") as ps_o,
            tc.tile_pool(name="ps_p", bufs=1, space="PSUM") as ps_p,
        ):
            # ---- constants / weights ----
            w1t = const.tile([128, 4, HG_CH], BF16)
            w2t = const.tile([128, 4, HG_CH], BF16)
            w3t = const.tile([128, 4, 260], BF16)
            b1c = const.tile([128, 2], F32)
            b2c = const.tile([128, 2], F32)
            b3r = const.tile([1, 260], BF16)
            onesc = const.tile([128, 1], BF16)
            ones128 = const.tile([1, 128], BF16)
            ones64 = const.tile([1, 64], F32)
            for sb, dr in (
                (w1t, w1t_d), (w2t, w2t_d), (w3t, w3t_d),
                (b1c, b1c_d), (b2c, b2c_d), (b3r, b3r_d),
                (onesc, ones_d), (ones128, ones128_d), (ones64, ones64_d),
            ):
                nc.sync.dma_start(out=sb[:], in_=dr[:])

            # ---- big resident tensors ----
            qts = big.tile([128, 4, NP], BF16)
            nc.sync.dma_start(out=qts[:], in_=qt_d[:])
            maskt = big.tile([128, NKC, NP], BF16)
            for i in range(8):
                nc.sync.dma_start(
                    out=maskt[:, 4 * i:4 * (i + 1), :],
                    in_=mask_d[:, 4 * i:4 * (i + 1), :],
                )

            qhT = big.tile([128, 2, NP], BF16)   # [ch_in_pair, pair, query]
            khT = big.tile([128, 2, MNP], BF16)  # [ch_in_pair, pair, key]
            vh = big.tile([128, NKC, 260], BF16)  # [key_in_chunk, kc, 4*(64ch+1)]

            # ---- qhT projection ----
            for pair in range(2):
                for nq in range(NP // 512):
                    pt = ps_p.tile([128, 512], F32, tag="pp")
                    for c in range(4):
                        nc.tensor.matmul(
                            pt[:],
                            lhsT=w1t[:, c, pair * 128:(pair + 1) * 128],
                            rhs=qts[:, c, nq * 512:(nq + 1) * 512],
                            start=(c == 0),
                            stop=(c == 3),
                        )
                    nc.vector.tensor_tensor(
                        out=qhT[:, pair, nq * 512:(nq + 1) * 512],
                        in0=pt[:],
                        in1=b1c[:, pair:pair + 1].to_broadcast((128, 512)),
                        op=ALU.add,
                    )

            # ---- khT / vh projections, interleaved with attention (qt0, hp0) ----
            def proj_k_chunk(nq):
                kts = kio.tile([128, 4, 512], BF16, tag="kts")
                nc.sync.dma_start(out=kts[:], in_=kt_d[:, :, nq * 512:(nq + 1) * 512])
                for pair in range(2):
                    pt = ps_p.tile([128, 512], F32, tag="pp")
                    for c in range(4):
                        nc.tensor.matmul(
                            pt[:],
                            lhsT=w2t[:, c, pair * 128:(pair + 1) * 128],
                            rhs=kts[:, c, :],
                            start=(c == 0),
                            stop=(c == 3),
                        )
                    nc.vector.tensor_tensor(
                        out=khT[:, pair, nq * 512:(nq + 1) * 512],
                        in0=pt[:],
                        in1=b2c[:, pair:pair + 1].to_broadcast((128, 512)),
                        op=ALU.add,
                    )

            def proj_v_chunk(kc):
                vts = vio.tile([128, 4, 128], BF16, tag="vts")
                nc.sync.dma_start(out=vts[:], in_=vt_d[:, :, kc * 128:(kc + 1) * 128])
                pt = ps_p.tile([128, 260], F32, tag="pp")
                for c in range(4):
                    nc.tensor.matmul(
                        pt[:],
                        lhsT=vts[:, c, :],
                        rhs=w3t[:, c, :],
                        start=(c == 0),
                        stop=False,
                    )
                nc.tensor.matmul(
                    pt[:], lhsT=ones128[:], rhs=b3r[:], start=False, stop=True,
                )
                nc.vector.tensor_copy(out=vh[:, kc, :], in_=pt[:])

            # attention inner body for one (qt, hp, kc)
            def attn_kc(qt, hp, kc, o_ps):
                s_ps = ps_s.tile([128, 1024], F32, tag="s")
                for h in range(2):  # head within pair: partitions h*64..h*64+63
                    nc.tensor.matmul(
                        s_ps[:, h * 512:(h + 1) * 512],
                        lhsT=khT[h * 64:(h + 1) * 64, hp, kc * 128:(kc + 1) * 128],
                        rhs=qhT[h * 64:(h + 1) * 64, hp, qt * 512:(qt + 1) * 512],
                        start=True,
                        stop=True,
                    )
                numer = work.tile([128, 1024], BF16, tag="numer")
                nc.scalar.activation(numer[:], s_ps[:], AF.Exp)
                numerm = work.tile([128, 1024], BF16, tag="numerm")
                for h in range(2):
                    nc.vector.tensor_tensor(
                        out=numerm[:, h * 512:(h + 1) * 512],
                        in0=numer[:, h * 512:(h + 1) * 512],
                        in1=maskt[:, kc, qt * 512:(qt + 1) * 512],
                        op=ALU.mult,
                    )
                for h in range(2):
                    ch = (2 * hp + h) * 65
                    nc.tensor.matmul(
                        o_ps[h][:],
                        lhsT=vh[:, kc, ch:ch + 65],
                        rhs=numerm[:, h * 512:(h + 1) * 512],
                        start=(kc == 0),
                        stop=(kc == NKC - 1),
                    )

            def attn_tail(qt, hp, o_ps):
                b_ps = ps_p.tile([128, 512], F32, tag="pp")
                o_sb = fin.tile([128, 512], F32, tag="osb")
                for h in range(2):
                    dsb = work.tile([1, 512], F32, tag=f"dsb{h}")
                    nc.vector.tensor_copy(out=dsb[:], in_=o_ps[h][64:65, :])
                    drow = work.tile([128, 4], F32, tag=f"drow{h}")
                    nc.sync.dma_start(out=drow[:], in_=dsb[:])
                    rrow = work.tile([128, 4], F32, tag=f"rrow{h}")
                    nc.vector.reciprocal(rrow[:], drow[:])
                    rec = work.tile([1, 512], F32, tag=f"rec{h}")
                    nc.sync.dma_start(out=rec[:], in_=rrow[:])
                    nc.tensor.matmul(
                        b_ps[h * 64:(h + 1) * 64, :],
                        lhsT=ones64[:],
                        rhs=rec[:],
                        start=True,
                        stop=True,
                    )
                    nc.vector.tensor_copy(
                        out=o_sb[h * 64:(h + 1) * 64, :], in_=o_ps[h][0:64, :]
                    )
                outt = fin.tile([128, 512], F32, tag="outt")
                nc.vector.tensor_tensor(
                    out=outt[:], in0=o_sb[:], in1=b_ps[:], op=ALU.mult
                )
                nc.sync.dma_start(out=out_d[hp, :, qt, :], in_=outt[:])

            # Interleave K/V projections with the first (qt0, hp0) attention pass
            # so ScalarE's exp stream starts early.
            o_ps0 = [ps_o.tile([65, 512], F32, tag=f"o{h}", name=f"ops00{h}") for h in range(2)]
            for nq in range(8):
                proj_k_chunk(nq)
                for kc in range(4 * nq, 4 * nq + 4):
                    proj_v_chunk(kc)
                if nq > 0:
                    for kc in range(4 * (nq - 1), 4 * nq):
                        attn_kc(0, 0, kc, o_ps0)
            for kc in range(28, 32):
                attn_kc(0, 0, kc, o_ps0)
            attn_tail(0, 0, o_ps0)

            for qt, hp in ((0, 1), (1, 0), (1, 1)):
                o_ps = [ps_o.tile([65, 512], F32, tag=f"o{h}", name=f"ops{qt}{hp}{h}") for h in range(2)]
                for kc in range(NKC):
                    attn_kc(qt, hp, kc, o_ps)
                attn_tail(qt, hp, o_ps)

    _split_multi_waits(nc)
    return nc


def _prep_inputs(q, k, v, rns_indices, W1, b1, W2, b2, W3, b3):
    bf = ml_dtypes.bfloat16
    q = np.asarray(q, np.float32)
    k = np.asarray(k, np.float32)
    v = np.asarray(v, np.float32)
    idx = np.asarray(rns_indices)
    W1 = np.asarray(W1, np.float32)
    W2 = np.asarray(W2, np.float32)
    W3 = np.asarray(W3, np.float32)
    b1 = np.asarray(b1, np.float32)
    b2 = np.asarray(b2, np.float32)
    b3 = np.asarray(b3, np.float32)
    scale = 1.0 / np.sqrt(DH)

    def part3(x2d, n):  # (512, n) -> (128, 4, n)
        return np.ascontiguousarray(
            x2d.reshape(4, 128, n).transpose(1, 0, 2)
        ).astype(bf)

    def _aug_w3(W3h):  # (256, 512) -> (128, 4, 260) with zero cols at ones slots
        wt = np.zeros((DIM, 260), np.float32)
        for h in range(4):
            wt[:, h * 65:h * 65 + 64] = W3h[h * 64:(h + 1) * 64, :].T
        return part3(wt, 260)

    def _aug_b3(b3h):  # (256,) -> (1, 260) with 1.0 at ones slots
        br = np.zeros((1, 260), np.float32)
        for h in range(4):
            br[0, h * 65:h * 65 + 64] = b3h[h * 64:(h + 1) * 64]
            br[0, h * 65 + 64] = 1.0
        return br.astype(bf)

    masks = []
    for b in range(BS):
        m = np.zeros((NP, MNP), np.float32)
        m[np.arange(NP)[:, None], idx[b]] = 1.0
        mt = m.T.reshape(NKC, 128, NP).transpose(1, 0, 2)
        masks.append(np.ascontiguousarray(mt).astype(bf))

    qkv_t = []
    for b in range(BS):
        qkv_t.append(
            (
                part3(q[:, b, :].T, NP),
                part3(k[:, b, :].T, MNP),
                part3(v[:, b, :].T, MNP),
            )
        )

    in_maps = []
    for core in range(N_CORES):
        b, hg = core // 2, core % 2
        sl = slice(hg * HG_CH, (hg + 1) * HG_CH)
        qtb, ktb, vtb = qkv_t[b]
        im = {
            "qt": qtb,
            "kt": ktb,
            "vt": vtb,
            "w1t": part3(W1[sl, :].T * scale, HG_CH),
            "w2t": part3(W2[sl, :].T, HG_CH),
            "w3t": _aug_w3(W3[sl, :]),
            "b1c": np.ascontiguousarray(
                (b1[sl] * scale).reshape(2, 128).T
            ).astype(np.float32),
            "b2c": np.ascontiguousarray(b2[sl].reshape(2, 128).T).astype(np.float32),
            "b3r": _aug_b3(b3[sl]),
            "maskt": masks[b],
            "onesc": np.ones((128, 1), bf),
            "ones128": np.ones((1, 128), bf),
            "ones64": np.ones((1, 64), np.float32),
        }
        in_maps.append(im)
    return in_maps


def kernel(q, k, v, rns_indices, W1, b1, W2, b2, W3, b3):
    nc = _build_nc()
    in_maps = _prep_inputs(q, k, v, rns_indices, W1, b1, W2, b2, W3, b3)
    res = run_bass_kernel_spmd(
        nc,
        in_maps,
        core_ids=list(range(N_CORES)),
        trace=run_opts["trace"],
        **run_opts["trace_kwargs"],
    )
    _last_results["res"] = res

    out = np.empty((NP, BS, DIM), np.float32)
    for core in range(N_CORES):
        b, hg = core // 2, core % 2
        r = np.asarray(res.results[core]["outt"], np.float32)  # (2,128,2,512)
        arr = r.transpose(2, 3, 0, 1).reshape(NP, HG_CH)
        out[:, b, hg * HG_CH:(hg + 1) * HG_CH] = arr
    return out



# revision 2
# speedup vs baseline: 1.1358x; 1.1358x over previous
"""Sparse cross-attention kernel for Trainium2 (8 NeuronCores, SPMD).

Problem: nn_CrossAttn (NP=1024 queries, MNP=4096 keys, BS=4, DIM=512,
NH=8 heads, dh=64, TOPK=32 sparse mask shared across heads).

Sharding: core = (batch b, head-group hg).  4 batches x 2 head-groups of 4
heads each.  Each core computes its batch's attention for its 4 heads and
writes a (256 ch, 1024 q) transposed output block; the host reassembles.

v2 layout: ScalarE exp of the dense masked scores is the hard floor
(~1.1us per (qt,hp,kc) unit, 128 units).  The kernel is a single software
pipeline that keeps ScalarE saturated from ~5us on:
  - JIT chunk DMAs (kt/vt/mask per kc; mask split by qt-half over passes 1-2)
  - projections interleaved across all 32 kc of pass 1
  - scores as a row-tiled concurrent pair (2 heads, K=64 each)
  - AV (den folded as 65th channel) issued one kc behind the score wave
  - tails release AV PSUM accumulators early (copies first, recip after)
"""

import numpy as np
import ml_dtypes

import concourse.bass as bass
import concourse.mybir as mybir
import concourse.tile as tile
from concourse.bass_utils import run_bass_kernel_spmd

BF16 = mybir.dt.bfloat16
F32 = mybir.dt.float32
AF = mybir.ActivationFunctionType
ALU = mybir.AluOpType

NH = 8
DIM = 512
NP = 1024
MNP = 4096
BS = 4
DH = 64
N_CORES = 8
HG_CH = 256          # channels per head-group (4 heads x 64)
NKC = MNP // 128     # 32 key chunks
NQT = NP // 512      # 2 query tiles

# options test.py can flip
run_opts = {"trace": False, "trace_kwargs": {}}
_last_results = {}


def _split_multi_waits(nc):
    """This container's walrus encodes only ONE sync-wait per TPB instruction
    (newer Tile emits several).  Split extras onto preceding NOPs."""
    eng_ok = {
        mybir.EngineType.PE,
        mybir.EngineType.Activation,
        mybir.EngineType.DVE,
        mybir.EngineType.Pool,
        mybir.EngineType.SP,
    }
    for fn in nc.m.functions:
        for blk in fn.blocks:
            insts = blk.instructions
            out = []
            changed = False
            for inst in insts:
                si = inst.sync_info
                if (
                    si is not None
                    and si.on_wait
                    and len(si.on_wait) > 1
                    and inst.engine in eng_ok
                ):
                    waits = list(si.on_wait)
                    for j, w in enumerate(waits[:-1]):
                        out.append(
                            mybir.InstNoOp(
                                name=f"{inst.name}-w{j}",
                                engine=inst.engine,
                                ins=[],
                                outs=[],
                                sync_info=mybir.SyncInfo(on_wait=[w], on_update=[]),
                            )
                        )
                    inst.sync_info = mybir.SyncInfo(
                        on_wait=[waits[-1]], on_update=list(si.on_update)
                    )
                    changed = True
                out.append(inst)
            if changed:
                blk.instructions = out


def _build_nc() -> bass.Bass:
    nc = bass.Bass()

    qt_d = nc.dram_tensor("qt", [128, 4, NP], BF16, kind="ExternalInput")
    kt_d = nc.dram_tensor("kt", [128, 4, MNP], BF16, kind="ExternalInput")
    vt_d = nc.dram_tensor("vt", [128, 4, MNP], BF16, kind="ExternalInput")
    w1t_d = nc.dram_tensor("w1t", [128, 4, HG_CH], BF16, kind="ExternalInput")
    w2t_d = nc.dram_tensor("w2t", [128, 4, HG_CH], BF16, kind="ExternalInput")
    w3t_d = nc.dram_tensor("w3t", [128, 4, 260], BF16, kind="ExternalInput")
    b1c_d = nc.dram_tensor("b1c", [128, 2], F32, kind="ExternalInput")
    b2c_d = nc.dram_tensor("b2c", [128, 2], F32, kind="ExternalInput")
    b3r_d = nc.dram_tensor("b3r", [1, 260], BF16, kind="ExternalInput")
    mask_d = nc.dram_tensor("maskt", [128, NKC, NP], BF16, kind="ExternalInput")
    ones_d = nc.dram_tensor("onesc", [128, 1], BF16, kind="ExternalInput")
    ones128_d = nc.dram_tensor("ones128", [1, 128], BF16, kind="ExternalInput")
    ones64_d = nc.dram_tensor("ones64", [1, 64], F32, kind="ExternalInput")
    out_d = nc.dram_tensor("outt", [2, 128, NQT, 512], F32, kind="ExternalOutput")

    with tile.TileContext(nc) as tc:
        with (
            tc.tile_pool(name="const", bufs=1) as const,
            tc.tile_pool(name="big", bufs=1) as big,
            tc.tile_pool(name="kio", bufs=3) as kio,
            tc.tile_pool(name="vio", bufs=3) as vio,
            tc.tile_pool(name="work", bufs=4) as work,
            tc.tile_pool(name="fin", bufs=2) as fin,
            tc.tile_pool(name="ps_s", bufs=2, space="PSUM") as ps_s,
            tc.tile_pool(name="ps_o", bufs=1, space="PSUM") as ps_o,
            tc.tile_pool(name="ps_p", bufs=2, space="PSUM") as ps_p,
        ):
            # ---- constants / weights (q-proj inputs first) ----
            w1t = const.tile([128, 4, HG_CH], BF16)
            b1c = const.tile([128, 2], F32)
            qts = big.tile([128, 4, NP], BF16)
            nc.sync.dma_start(out=w1t[:], in_=w1t_d[:])
            nc.sync.dma_start(out=b1c[:], in_=b1c_d[:])
            nc.sync.dma_start(out=qts[:], in_=qt_d[:])

            w2t = const.tile([128, 4, HG_CH], BF16)
            w3t = const.tile([128, 4, 260], BF16)
            b2c = const.tile([128, 2], F32)
            b3r = const.tile([1, 260], BF16)
            onesc = const.tile([128, 1], BF16)
            ones128 = const.tile([1, 128], BF16)
            ones64 = const.tile([1, 64], F32)
            for sb, dr in (
                (w2t, w2t_d), (w3t, w3t_d), (b2c, b2c_d), (b3r, b3r_d),
                (onesc, ones_d), (ones128, ones128_d), (ones64, ones64_d),
            ):
                nc.sync.dma_start(out=sb[:], in_=dr[:])

            maskt = big.tile([128, NKC, NP], BF16)
            qhT = big.tile([128, 2, NP], BF16)   # [ch_in_pair, pair, query]
            khT = big.tile([128, 2, MNP], BF16)  # [ch_in_pair, pair, key]
            vh = big.tile([128, NKC, 260], BF16)  # [key_in_chunk, kc, 4*(64ch+1)]

            # ---- qhT projection ----
            for pair in range(2):
                for nq in range(NP // 512):
                    pt = ps_p.tile([128, 512], F32, tag="pp")
                    for c in range(4):
                        nc.tensor.matmul(
                            pt[:],
                            lhsT=w1t[:, c, pair * 128:(pair + 1) * 128],
                            rhs=qts[:, c, nq * 512:(nq + 1) * 512],
                            start=(c == 0),
                            stop=(c == 3),
                        )
                    nc.vector.tensor_tensor(
                        out=qhT[:, pair, nq * 512:(nq + 1) * 512],
                        in0=pt[:],
                        in1=b1c[:, pair:pair + 1].to_broadcast((128, 512)),
                        op=ALU.add,
                    )

            def proj_k_chunk(nq):
                kts = kio.tile([128, 4, 512], BF16, tag="kts")
                nc.sync.dma_start(out=kts[:], in_=kt_d[:, :, nq * 512:(nq + 1) * 512])
                for pair in range(2):
                    pt = ps_p.tile([128, 512], F32, tag="pp")
                    for c in range(4):
                        nc.tensor.matmul(
                            pt[:],
                            lhsT=w2t[:, c, pair * 128:(pair + 1) * 128],
                            rhs=kts[:, c, :],
                            start=(c == 0),
                            stop=(c == 3),
                        )
                    nc.vector.tensor_tensor(
                        out=khT[:, pair, nq * 512:(nq + 1) * 512],
                        in0=pt[:],
                        in1=b2c[:, pair:pair + 1].to_broadcast((128, 512)),
                        op=ALU.add,
                    )

            def proj_v_chunk(kc):
                vts = vio.tile([128, 4, 128], BF16, tag="vts")
                nc.sync.dma_start(out=vts[:], in_=vt_d[:, :, kc * 128:(kc + 1) * 128])
                pt = ps_p.tile([128, 260], F32, tag="pp")
                for c in range(4):
                    nc.tensor.matmul(
                        pt[:],
                        lhsT=vts[:, c, :],
                        rhs=w3t[:, c, :],
                        start=(c == 0),
                        stop=False,
                    )
                nc.tensor.matmul(
                    pt[:], lhsT=ones128[:], rhs=b3r[:], start=False, stop=True,
                )
                nc.vector.tensor_copy(out=vh[:, kc, :], in_=pt[:])

            def issue_av(qt, hp, kc, numerm, o_ps):
                for h in range(2):
                    ch = (2 * hp + h) * 65
                    nc.tensor.matmul(
                        o_ps[h][:],
                        lhsT=vh[:, kc, ch:ch + 65],
                        rhs=numerm[:, h, :],
                        start=(kc == 0),
                        stop=(kc == NKC - 1),
                    )

            def attn_tail(qt, hp, o_ps):
                # Copies first: they are the last readers of o_ps, so the next
                # pass's AV accumulators can start while the recip path runs.
                o_sb = fin.tile([128, 512], F32, tag="osb")
                dsbs = []
                for h in range(2):
                    dsb = work.tile([1, 512], F32, tag=f"dsb{h}")
                    nc.vector.tensor_copy(out=dsb[:], in_=o_ps[h][64:65, :])
                    nc.vector.tensor_copy(
                        out=o_sb[h * 64:(h + 1) * 64, :], in_=o_ps[h][0:64, :]
                    )
                    dsbs.append(dsb)
                b_ps = ps_p.tile([128, 512], F32, tag="pp")
                for h in range(2):
                    drow = work.tile([128, 4], F32, tag=f"drow{h}")
                    nc.sync.dma_start(out=drow[:], in_=dsbs[h][:])
                    rrow = work.tile([128, 4], F32, tag=f"rrow{h}")
                    nc.vector.reciprocal(rrow[:], drow[:])
                    rec = work.tile([1, 512], F32, tag=f"rec{h}")
                    nc.sync.dma_start(out=rec[:], in_=rrow[:])
                    nc.tensor.matmul(
                        b_ps[h * 64:(h + 1) * 64, :],
                        lhsT=ones64[:],
                        rhs=rec[:],
                        start=True,
                        stop=True,
                    )
                outt = fin.tile([128, 512], F32, tag="outt")
                nc.vector.tensor_tensor(
                    out=outt[:], in0=o_sb[:], in1=b_ps[:], op=ALU.mult
                )
                nc.sync.dma_start(out=out_d[hp, :, qt, :], in_=outt[:])

            # ---- main pipeline: 4 passes over (qt, hp), proj in pass 0 ----
            for pi, (qt, hp) in enumerate(((0, 0), (0, 1), (1, 0), (1, 1))):
                o_ps = [
                    ps_o.tile([65, 512], F32, tag=f"o{h}", name=f"ops{qt}{hp}{h}")
                    for h in range(2)
                ]
                pend = None
                for kc in range(NKC):
                    if pi == 0:
                        if kc % 4 == 0:
                            proj_k_chunk(kc // 4)
                        proj_v_chunk(kc)
                        nc.sync.dma_start(
                            out=maskt[:, kc, 0:512], in_=mask_d[:, kc, 0:512]
                        )
                    elif pi == 1:
                        nc.sync.dma_start(
                            out=maskt[:, kc, 512:1024], in_=mask_d[:, kc, 512:1024]
                        )
                    # scores: row-tiled concurrent pair (K=64 each)
                    s = ps_s.tile([128, 2, 512], F32, tag="s")
                    for h in range(2):
                        nc.tensor.matmul(
                            s[:, h, :],
                            lhsT=khT[h * 64:(h + 1) * 64, hp, kc * 128:(kc + 1) * 128],
                            rhs=qhT[h * 64:(h + 1) * 64, hp, qt * 512:(qt + 1) * 512],
                            start=True,
                            stop=True,
                        )
                    # AV for the previous kc after this wave: keeps PE fed
                    # while ScalarE/VectorE process this kc.
                    if pend is not None:
                        issue_av(qt, hp, pend[0], pend[1], o_ps)
                    numer = work.tile([128, 2, 512], BF16, tag="numer")
                    nc.scalar.activation(numer[:], s[:], AF.Exp)
                    numerm = work.tile([128, 2, 512], BF16, tag="numerm")
                    nc.vector.tensor_tensor(
                        out=numerm[:],
                        in0=numer[:],
                        in1=maskt[:, kc:kc + 1, qt * 512:(qt + 1) * 512].to_broadcast(
                            (128, 2, 512)
                        ),
                        op=ALU.mult,
                    )
                    pend = (kc, numerm)
                issue_av(qt, hp, pend[0], pend[1], o_ps)
                attn_tail(qt, hp, o_ps)

    _split_multi_waits(nc)
    return nc


def _prep_inputs(q, k, v, rns_indices, W1, b1, W2, b2, W3, b3):
    bf = ml_dtypes.bfloat16
    q = np.asarray(q, np.float32)
    k = np.asarray(k, np.float32)
    v = np.asarray(v, np.float32)
    idx = np.asarray(rns_indices)
    W1 = np.asarray(W1, np.float32)
    W2 = np.asarray(W2, np.float32)
    W3 = np.asarray(W3, np.float32)
    b1 = np.asarray(b1, np.float32)
    b2 = np.asarray(b2, np.float32)
    b3 = np.asarray(b3, np.float32)
    scale = 1.0 / np.sqrt(DH)

    def part3(x2d, n):  # (512, n) -> (128, 4, n)
        return np.ascontiguousarray(
            x2d.reshape(4, 128, n).transpose(1, 0, 2)
        ).astype(bf)

    def _aug_w3(W3h):  # (256, 512) -> (128, 4, 260) with zero cols at ones slots
        wt = np.zeros((DIM, 260), np.float32)
        for h in range(4):
            wt[:, h * 65:h * 65 + 64] = W3h[h * 64:(h + 1) * 64, :].T
        return part3(wt, 260)

    def _aug_b3(b3h):  # (256,) -> (1, 260) with 1.0 at ones slots
        br = np.zeros((1, 260), np.float32)
        for h in range(4):
            br[0, h * 65:h * 65 + 64] = b3h[h * 64:(h + 1) * 64]
            br[0, h * 65 + 64] = 1.0
        return br.astype(bf)

    masks = []
    for b in range(BS):
        m = np.zeros((NP, MNP), np.float32)
        m[np.arange(NP)[:, None], idx[b]] = 1.0
        mt = m.T.reshape(NKC, 128, NP).transpose(1, 0, 2)
        masks.append(np.ascontiguousarray(mt).astype(bf))

    qkv_t = []
    for b in range(BS):
        qkv_t.append(
            (
                part3(q[:, b, :].T, NP),
                part3(k[:, b, :].T, MNP),
                part3(v[:, b, :].T, MNP),
            )
        )

    in_maps = []
    for core in range(N_CORES):
        b, hg = core // 2, core % 2
        sl = slice(hg * HG_CH, (hg + 1) * HG_CH)
        qtb, ktb, vtb = qkv_t[b]
        im = {
            "qt": qtb,
            "kt": ktb,
            "vt": vtb,
            "w1t": part3(W1[sl, :].T * scale, HG_CH),
            "w2t": part3(W2[sl, :].T, HG_CH),
            "w3t": _aug_w3(W3[sl, :]),
            "b1c": np.ascontiguousarray(
                (b1[sl] * scale).reshape(2, 128).T
            ).astype(np.float32),
            "b2c": np.ascontiguousarray(b2[sl].reshape(2, 128).T).astype(np.float32),
            "b3r": _aug_b3(b3[sl]),
            "maskt": masks[b],
            "onesc": np.ones((128, 1), bf),
            "ones128": np.ones((1, 128), bf),
            "ones64": np.ones((1, 64), np.float32),
        }
        in_maps.append(im)
    return in_maps


def kernel(q, k, v, rns_indices, W1, b1, W2, b2, W3, b3):
    nc = _build_nc()
    in_maps = _prep_inputs(q, k, v, rns_indices, W1, b1, W2, b2, W3, b3)
    res = run_bass_kernel_spmd(
        nc,
        in_maps,
        core_ids=list(range(N_CORES)),
        trace=run_opts["trace"],
        **run_opts["trace_kwargs"],
    )
    _last_results["res"] = res

    out = np.empty((NP, BS, DIM), np.float32)
    for core in range(N_CORES):
        b, hg = core // 2, core % 2
        r = np.asarray(res.results[core]["outt"], np.float32)  # (2,128,2,512)
        arr = r.transpose(2, 3, 0, 1).reshape(NP, HG_CH)
        out[:, b, hg * HG_CH:(hg + 1) * HG_CH] = arr
    return out


# revision 6
# speedup vs baseline: 1.1893x; 1.0471x over previous
"""Sparse cross-attention kernel for Trainium2 (8 NeuronCores, SPMD).

Problem: nn_CrossAttn (NP=1024 queries, MNP=4096 keys, BS=4, DIM=512,
NH=8 heads, dh=64, TOPK=32 sparse mask shared across heads).

Sharding: core = (batch b, head-group hg).  4 batches x 2 head-groups of 4
heads each.  Each core computes its batch's attention for its 4 heads and
writes a (256 ch, 1024 q) transposed output block; the host reassembles.

v2 layout: ScalarE exp of the dense masked scores is the hard floor
(~1.1us per (qt,hp,kc) unit, 128 units).  The kernel is a single software
pipeline that keeps ScalarE saturated from ~5us on:
  - JIT chunk DMAs (kt/vt/mask per kc; mask split by qt-half over passes 1-2)
  - projections interleaved across all 32 kc of pass 1
  - scores as a row-tiled concurrent pair (2 heads, K=64 each)
  - AV (den folded as 65th channel) issued one kc behind the score wave
  - tails release AV PSUM accumulators early (copies first, recip after)
"""

import numpy as np
import ml_dtypes

import concourse.bass as bass
import concourse.mybir as mybir
import concourse.tile as tile
from concourse.bass_utils import run_bass_kernel_spmd

BF16 = mybir.dt.bfloat16
F32 = mybir.dt.float32
AF = mybir.ActivationFunctionType
ALU = mybir.AluOpType

NH = 8
DIM = 512
NP = 1024
MNP = 4096
BS = 4
DH = 64
N_CORES = 8
HG_CH = 256          # channels per head-group (4 heads x 64)
NKC = MNP // 128     # 32 key chunks
NQT = NP // 512      # 2 query tiles

# options test.py can flip
run_opts = {"trace": False, "trace_kwargs": {}}
_last_results = {}


def _split_multi_waits(nc):
    """This container's walrus encodes only ONE sync-wait per TPB instruction
    (newer Tile emits several).  Split extras onto preceding NOPs."""
    eng_ok = {
        mybir.EngineType.PE,
        mybir.EngineType.Activation,
        mybir.EngineType.DVE,
        mybir.EngineType.Pool,
        mybir.EngineType.SP,
    }
    for fn in nc.m.functions:
        for blk in fn.blocks:
            insts = blk.instructions
            out = []
            changed = False
            for inst in insts:
                si = inst.sync_info
                if (
                    si is not None
                    and si.on_wait
                    and len(si.on_wait) > 1
                    and inst.engine in eng_ok
                ):
                    waits = list(si.on_wait)
                    for j, w in enumerate(waits[:-1]):
                        out.append(
                            mybir.InstNoOp(
                                name=f"{inst.name}-w{j}",
                                engine=inst.engine,
                                ins=[],
                                outs=[],
                                sync_info=mybir.SyncInfo(on_wait=[w], on_update=[]),
                            )
                        )
                    inst.sync_info = mybir.SyncInfo(
                        on_wait=[waits[-1]], on_update=list(si.on_update)
                    )
                    changed = True
                out.append(inst)
            if changed:
                blk.instructions = out


def _build_nc() -> bass.Bass:
    nc = bass.Bass()

    qt_d = nc.dram_tensor("qt", [128, 4, NP], BF16, kind="ExternalInput")
    kt_d = nc.dram_tensor("kt", [128, 4, MNP], BF16, kind="ExternalInput")
    vt_d = nc.dram_tensor("vt", [128, 4, MNP], BF16, kind="ExternalInput")
    w1t_d = nc.dram_tensor("w1t", [128, 4, HG_CH], BF16, kind="ExternalInput")
    w2t_d = nc.dram_tensor("w2t", [128, 4, HG_CH], BF16, kind="ExternalInput")
    w3t_d = nc.dram_tensor("w3t", [128, 4, 260], BF16, kind="ExternalInput")
    b1c_d = nc.dram_tensor("b1c", [128, 2], F32, kind="ExternalInput")
    b2c_d = nc.dram_tensor("b2c", [128, 2], F32, kind="ExternalInput")
    b3r_d = nc.dram_tensor("b3r", [1, 260], BF16, kind="ExternalInput")
    mask_d = nc.dram_tensor("maskt", [128, NKC, NP], BF16, kind="ExternalInput")
    ones_d = nc.dram_tensor("onesc", [128, 1], BF16, kind="ExternalInput")
    ones128_d = nc.dram_tensor("ones128", [1, 128], BF16, kind="ExternalInput")
    ones64_d = nc.dram_tensor("ones64", [1, 64], F32, kind="ExternalInput")
    out_d = nc.dram_tensor("outt", [2, 128, NQT, 512], F32, kind="ExternalOutput")

    with tile.TileContext(nc) as tc:
        with (
            tc.tile_pool(name="const", bufs=1) as const,
            tc.tile_pool(name="big", bufs=1) as big,
            tc.tile_pool(name="kio", bufs=3) as kio,
            tc.tile_pool(name="vio", bufs=3) as vio,
            tc.tile_pool(name="work", bufs=4) as work,
            tc.tile_pool(name="fin", bufs=2) as fin,
            tc.tile_pool(name="ps_s", bufs=2, space="PSUM") as ps_s,
            tc.tile_pool(name="ps_o", bufs=1, space="PSUM") as ps_o,
            tc.tile_pool(name="ps_p", bufs=2, space="PSUM") as ps_p,
        ):
            # ---- constants / weights (q-proj inputs first) ----
            w1t = const.tile([128, 4, HG_CH], BF16)
            b1c = const.tile([128, 2], F32)
            qts = big.tile([128, 4, NP], BF16)
            nc.sync.dma_start(out=w1t[:], in_=w1t_d[:])
            nc.sync.dma_start(out=b1c[:], in_=b1c_d[:])
            nc.sync.dma_start(out=qts[:], in_=qt_d[:])

            w2t = const.tile([128, 4, HG_CH], BF16)
            w3t = const.tile([128, 4, 260], BF16)
            b2c = const.tile([128, 2], F32)
            b3r = const.tile([1, 260], BF16)
            onesc = const.tile([128, 1], BF16)
            ones128 = const.tile([1, 128], BF16)
            ones64 = const.tile([1, 64], F32)
            for sb, dr in (
                (w2t, w2t_d), (w3t, w3t_d), (b2c, b2c_d), (b3r, b3r_d),
                (onesc, ones_d), (ones128, ones128_d), (ones64, ones64_d),
            ):
                nc.sync.dma_start(out=sb[:], in_=dr[:])

            maskt = big.tile([128, NKC, NP], BF16)
            qhT = big.tile([128, 2, NP], BF16)   # [ch_in_pair, pair, query]
            khT = big.tile([128, 2, MNP], BF16)  # [ch_in_pair, pair, key]
            vh = big.tile([128, NKC, 260], BF16)  # [key_in_chunk, kc, 4*(64ch+1)]

            # ---- qhT projection ----
            for pair in range(2):
                for nq in range(NP // 512):
                    pt = ps_p.tile([128, 512], F32, tag="pp")
                    for c in range(4):
                        nc.tensor.matmul(
                            pt[:],
                            lhsT=w1t[:, c, pair * 128:(pair + 1) * 128],
                            rhs=qts[:, c, nq * 512:(nq + 1) * 512],
                            start=(c == 0),
                            stop=(c == 3),
                        )
                    nc.scalar.activation(
                        qhT[:, pair, nq * 512:(nq + 1) * 512],
                        pt[:],
                        AF.Identity,
                        bias=b1c[:, pair:pair + 1],
                    )

            # staged k/v chunks, 4 kc (512 keys) per DMA; ring dicts keyed by nq
            k_stage = {}
            v_stage = {}

            def fetch_kv(nq):
                kts = kio.tile([128, 4, 512], BF16, tag="kts", name=f"kts{nq}")
                nc.sync.dma_start(out=kts[:], in_=kt_d[:, :, nq * 512:(nq + 1) * 512])
                k_stage[nq] = kts
                vts = vio.tile([128, 4, 512], BF16, tag="vts", name=f"vts{nq}")
                nc.sync.dma_start(out=vts[:], in_=vt_d[:, :, nq * 512:(nq + 1) * 512])
                v_stage[nq] = vts

            def proj_k_chunk(nq):
                kts = k_stage.pop(nq)
                for pair in range(2):
                    pt = ps_p.tile([128, 512], F32, tag="pp")
                    for c in range(4):
                        nc.tensor.matmul(
                            pt[:],
                            lhsT=w2t[:, c, pair * 128:(pair + 1) * 128],
                            rhs=kts[:, c, :],
                            start=(c == 0),
                            stop=(c == 3),
                        )
                    # eviction + bias on ScalarE: pass 0 is PE-bound, ScalarE
                    # has slack there, and this keeps the DVE queue short.
                    nc.scalar.activation(
                        khT[:, pair, nq * 512:(nq + 1) * 512],
                        pt[:],
                        AF.Identity,
                        bias=b2c[:, pair:pair + 1],
                    )

            def proj_v_chunk(kc):
                vts = v_stage[kc // 4]
                co = (kc % 4) * 128
                pt = ps_p.tile([128, 260], F32, tag="pp")
                for c in range(4):
                    nc.tensor.matmul(
                        pt[:],
                        lhsT=vts[:, c, co:co + 128],
                        rhs=w3t[:, c, :],
                        start=(c == 0),
                        stop=False,
                    )
                nc.tensor.matmul(
                    pt[:], lhsT=ones128[:], rhs=b3r[:], start=False, stop=True,
                )
                nc.scalar.copy(out=vh[:, kc, :], in_=pt[:])
                if kc % 4 == 3:
                    del v_stage[kc // 4]

            def issue_av(qt, hp, kc, numerm, o_ps):
                for h in range(2):
                    ch = (2 * hp + h) * 65
                    nc.tensor.matmul(
                        o_ps[h][:],
                        lhsT=vh[:, kc, ch:ch + 65],
                        rhs=numerm[:, h, :],
                        start=(kc == 0),
                        stop=(kc == NKC - 1),
                    )

            def attn_tail_a(qt, hp, o_ps):
                # Copies only: they are the last readers of o_ps, so the next
                # pass's AV accumulators can start while the recip path runs.
                o_sb = fin.tile([128, 512], F32, tag="osb")
                dsbs = []
                for h in range(2):
                    dsb = work.tile([1, 512], F32, tag=f"dsb{h}")
                    nc.vector.tensor_copy(out=dsb[:], in_=o_ps[h][64:65, :])
                    nc.vector.tensor_copy(
                        out=o_sb[h * 64:(h + 1) * 64, :], in_=o_ps[h][0:64, :]
                    )
                    dsbs.append(dsb)
                return o_sb, dsbs

            def attn_tail_b(qt, hp, o_sb, dsbs):
                # Deferred into the next pass so the recip DMA round-trips and
                # the b_ps matmuls never stall the main PE stream.
                b_ps = ps_p.tile([128, 512], F32, tag="pp")
                for h in range(2):
                    drow = work.tile([128, 4], F32, tag=f"drow{h}")
                    nc.sync.dma_start(out=drow[:], in_=dsbs[h][:])
                    rrow = work.tile([128, 4], F32, tag=f"rrow{h}")
                    nc.vector.reciprocal(rrow[:], drow[:])
                    rec = work.tile([1, 512], F32, tag=f"rec{h}")
                    nc.sync.dma_start(out=rec[:], in_=rrow[:])
                    nc.tensor.matmul(
                        b_ps[h * 64:(h + 1) * 64, :],
                        lhsT=ones64[:],
                        rhs=rec[:],
                        start=True,
                        stop=True,
                    )
                outt = fin.tile([128, 512], F32, tag="outt")
                nc.vector.tensor_tensor(
                    out=outt[:], in0=o_sb[:], in1=b_ps[:], op=ALU.mult
                )
                nc.sync.dma_start(out=out_d[hp, :, qt, :], in_=outt[:])

            # ---- main pipeline: 4 passes over (qt, hp), proj in pass 0 ----
            def fetch_mask(nq, half):
                nc.sync.dma_start(
                    out=maskt[:, 4 * nq:4 * (nq + 1), half * 512:(half + 1) * 512],
                    in_=mask_d[:, 4 * nq:4 * (nq + 1), half * 512:(half + 1) * 512],
                )

            pend_tail = None
            for pi, (qt, hp) in enumerate(((0, 0), (0, 1), (1, 0), (1, 1))):
                o_ps = [
                    ps_o.tile([65, 512], F32, tag=f"o{h}", name=f"ops{qt}{hp}{h}")
                    for h in range(2)
                ]
                pend = []
                for kc in range(NKC):
                    nq = kc // 4
                    if pi == 0:
                        if kc == 0:
                            fetch_kv(0)
                            fetch_mask(0, 0)
                            fetch_kv(1)
                            fetch_mask(1, 0)
                        if kc % 4 == 0:
                            if nq + 2 < 8:
                                fetch_kv(nq + 2)
                                fetch_mask(nq + 2, 0)
                            proj_k_chunk(nq)
                        proj_v_chunk(kc)
                    elif pi == 1 and kc % 4 == 0:
                        fetch_mask(nq, 1)
                    # scores: row-tiled concurrent pair (K=64 each)
                    s = ps_s.tile([128, 2, 512], F32, tag="s")
                    for h in range(2):
                        nc.tensor.matmul(
                            s[:, h, :],
                            lhsT=khT[h * 64:(h + 1) * 64, hp, kc * 128:(kc + 1) * 128],
                            rhs=qhT[h * 64:(h + 1) * 64, hp, qt * 512:(qt + 1) * 512],
                            start=True,
                            stop=True,
                        )
                    # deferred previous-pass finisher: PE is rolling again now
                    if kc == 3 and pend_tail is not None:
                        attn_tail_b(*pend_tail)
                        pend_tail = None
                    # AV lags the score wave by 2 kc to absorb exp/mask jitter
                    if len(pend) == 2:
                        issue_av(qt, hp, *pend.pop(0), o_ps)
                    numer = work.tile([128, 2, 512], BF16, tag="numer")
                    nc.scalar.activation(numer[:], s[:], AF.Exp)
                    numerm = work.tile([128, 2, 512], BF16, tag="numerm")
                    nc.vector.tensor_tensor(
                        out=numerm[:],
                        in0=numer[:],
                        in1=maskt[:, kc:kc + 1, qt * 512:(qt + 1) * 512].to_broadcast(
                            (128, 2, 512)
                        ),
                        op=ALU.mult,
                    )
                    pend.append((kc, numerm))
                for p in pend:
                    issue_av(qt, hp, *p, o_ps)
                o_sb, dsbs = attn_tail_a(qt, hp, o_ps)
                pend_tail = (qt, hp, o_sb, dsbs)
            attn_tail_b(*pend_tail)

    _split_multi_waits(nc)
    return nc


def _prep_inputs(q, k, v, rns_indices, W1, b1, W2, b2, W3, b3):
    bf = ml_dtypes.bfloat16
    q = np.asarray(q, np.float32)
    k = np.asarray(k, np.float32)
    v = np.asarray(v, np.float32)
    idx = np.asarray(rns_indices)
    W1 = np.asarray(W1, np.float32)
    W2 = np.asarray(W2, np.float32)
    W3 = np.asarray(W3, np.float32)
    b1 = np.asarray(b1, np.float32)
    b2 = np.asarray(b2, np.float32)
    b3 = np.asarray(b3, np.float32)
    scale = 1.0 / np.sqrt(DH)

    def part3(x2d, n):  # (512, n) -> (128, 4, n)
        return np.ascontiguousarray(
            x2d.reshape(4, 128, n).transpose(1, 0, 2)
        ).astype(bf)

    def _aug_w3(W3h):  # (256, 512) -> (128, 4, 260) with zero cols at ones slots
        wt = np.zeros((DIM, 260), np.float32)
        for h in range(4):
            wt[:, h * 65:h * 65 + 64] = W3h[h * 64:(h + 1) * 64, :].T
        return part3(wt, 260)

    def _aug_b3(b3h):  # (256,) -> (1, 260) with 1.0 at ones slots
        br = np.zeros((1, 260), np.float32)
        for h in range(4):
            br[0, h * 65:h * 65 + 64] = b3h[h * 64:(h + 1) * 64]
            br[0, h * 65 + 64] = 1.0
        return br.astype(bf)

    masks = []
    for b in range(BS):
        m = np.zeros((NP, MNP), np.float32)
        m[np.arange(NP)[:, None], idx[b]] = 1.0
        mt = m.T.reshape(NKC, 128, NP).transpose(1, 0, 2)
        masks.append(np.ascontiguousarray(mt).astype(bf))

    qkv_t = []
    for b in range(BS):
        qkv_t.append(
            (
                part3(q[:, b, :].T, NP),
                part3(k[:, b, :].T, MNP),
                part3(v[:, b, :].T, MNP),
            )
        )

    in_maps = []
    for core in range(N_CORES):
        b, hg = core // 2, core % 2
        sl = slice(hg * HG_CH, (hg + 1) * HG_CH)
        qtb, ktb, vtb = qkv_t[b]
        im = {
            "qt": qtb,
            "kt": ktb,
            "vt": vtb,
            "w1t": part3(W1[sl, :].T * scale, HG_CH),
            "w2t": part3(W2[sl, :].T, HG_CH),
            "w3t": _aug_w3(W3[sl, :]),
            "b1c": np.ascontiguousarray(
                (b1[sl] * scale).reshape(2, 128).T
            ).astype(np.float32),
            "b2c": np.ascontiguousarray(b2[sl].reshape(2, 128).T).astype(np.float32),
            "b3r": _aug_b3(b3[sl]),
            "maskt": masks[b],
            "onesc": np.ones((128, 1), bf),
            "ones128": np.ones((1, 128), bf),
            "ones64": np.ones((1, 64), np.float32),
        }
        in_maps.append(im)
    return in_maps


def kernel(q, k, v, rns_indices, W1, b1, W2, b2, W3, b3):
    nc = _build_nc()
    in_maps = _prep_inputs(q, k, v, rns_indices, W1, b1, W2, b2, W3, b3)
    res = run_bass_kernel_spmd(
        nc,
        in_maps,
        core_ids=list(range(N_CORES)),
        trace=run_opts["trace"],
        **run_opts["trace_kwargs"],
    )
    _last_results["res"] = res

    out = np.empty((NP, BS, DIM), np.float32)
    for core in range(N_CORES):
        b, hg = core // 2, core % 2
        r = np.asarray(res.results[core]["outt"], np.float32)  # (2,128,2,512)
        arr = r.transpose(2, 3, 0, 1).reshape(NP, HG_CH)
        out[:, b, hg * HG_CH:(hg + 1) * HG_CH] = arr
    return out


# revision 14
# speedup vs baseline: 1.2018x; 1.0105x over previous
"""Sparse cross-attention kernel for Trainium2 (8 NeuronCores, SPMD).

Problem: nn_CrossAttn (NP=1024 queries, MNP=4096 keys, BS=4, DIM=512,
NH=8 heads, dh=64, TOPK=32 sparse mask shared across heads).

Sharding: core = (batch b, head-group hg).  4 batches x 2 head-groups of 4
heads each.  Each core computes its batch's attention for its 4 heads and
writes a (256 ch, 1024 q) transposed output block; the host reassembles.

v2 layout: ScalarE exp of the dense masked scores is the hard floor
(~1.1us per (qt,hp,kc) unit, 128 units).  The kernel is a single software
pipeline that keeps ScalarE saturated from ~5us on:
  - JIT chunk DMAs (kt/vt/mask per kc; mask split by qt-half over passes 1-2)
  - projections interleaved across all 32 kc of pass 1
  - scores as a row-tiled concurrent pair (2 heads, K=64 each)
  - AV (den folded as 65th channel) issued one kc behind the score wave
  - tails release AV PSUM accumulators early (copies first, recip after)
"""

import numpy as np
import ml_dtypes

import concourse.bass as bass
import concourse.mybir as mybir
import concourse.tile as tile
from concourse.bass_utils import run_bass_kernel_spmd

BF16 = mybir.dt.bfloat16
F32 = mybir.dt.float32
AF = mybir.ActivationFunctionType
ALU = mybir.AluOpType

NH = 8
DIM = 512
NP = 1024
MNP = 4096
BS = 4
DH = 64
N_CORES = 8
HG_CH = 256          # channels per head-group (4 heads x 64)
NKC = MNP // 128     # 32 key chunks
NQT = NP // 512      # 2 query tiles

# options test.py can flip
run_opts = {"trace": False, "trace_kwargs": {}}
_last_results = {}


def _split_multi_waits(nc):
    """This container's walrus encodes only ONE sync-wait per TPB instruction
    (newer Tile emits several).  Split extras onto preceding NOPs."""
    eng_ok = {
        mybir.EngineType.PE,
        mybir.EngineType.Activation,
        mybir.EngineType.DVE,
        mybir.EngineType.Pool,
        mybir.EngineType.SP,
    }
    for fn in nc.m.functions:
        for blk in fn.blocks:
            insts = blk.instructions
            out = []
            changed = False
            for inst in insts:
                si = inst.sync_info
                if (
                    si is not None
                    and si.on_wait
                    and len(si.on_wait) > 1
                    and inst.engine in eng_ok
                ):
                    waits = list(si.on_wait)
                    for j, w in enumerate(waits[:-1]):
                        out.append(
                            mybir.InstNoOp(
                                name=f"{inst.name}-w{j}",
                                engine=inst.engine,
                                ins=[],
                                outs=[],
                                sync_info=mybir.SyncInfo(on_wait=[w], on_update=[]),
                            )
                        )
                    inst.sync_info = mybir.SyncInfo(
                        on_wait=[waits[-1]], on_update=list(si.on_update)
                    )
                    changed = True
                out.append(inst)
            if changed:
                blk.instructions = out


def _build_nc() -> bass.Bass:
    nc = bass.Bass()

    qt_d = nc.dram_tensor("qt", [128, 4, NP], BF16, kind="ExternalInput")
    kt_d = nc.dram_tensor("kt", [128, 4, MNP], BF16, kind="ExternalInput")
    vt_d = nc.dram_tensor("vt", [128, 4, MNP], BF16, kind="ExternalInput")
    w1t_d = nc.dram_tensor("w1t", [128, 4, HG_CH], BF16, kind="ExternalInput")
    w2t_d = nc.dram_tensor("w2t", [128, 4, HG_CH], BF16, kind="ExternalInput")
    w3t_d = nc.dram_tensor("w3t", [128, 4, 260], BF16, kind="ExternalInput")
    b1c_d = nc.dram_tensor("b1c", [128, 2], F32, kind="ExternalInput")
    b2c_d = nc.dram_tensor("b2c", [128, 2], F32, kind="ExternalInput")
    b3r_d = nc.dram_tensor("b3r", [1, 260], BF16, kind="ExternalInput")
    mask_d = nc.dram_tensor("maskt", [128, NKC, NP], BF16, kind="ExternalInput")
    ones_d = nc.dram_tensor("onesc", [128, 1], BF16, kind="ExternalInput")
    ones128_d = nc.dram_tensor("ones128", [1, 128], BF16, kind="ExternalInput")
    ones64_d = nc.dram_tensor("ones64", [1, 64], F32, kind="ExternalInput")
    out_d = nc.dram_tensor("outt", [2, 128, NQT, 512], F32, kind="ExternalOutput")

    with tile.TileContext(nc) as tc:
        with (
            tc.tile_pool(name="const", bufs=1) as const,
            tc.tile_pool(name="big", bufs=1) as big,
            tc.tile_pool(name="kio", bufs=3) as kio,
            tc.tile_pool(name="vio", bufs=3) as vio,
            tc.tile_pool(name="work", bufs=4) as work,
            tc.tile_pool(name="fin", bufs=2) as fin,
            tc.tile_pool(name="ps_s", bufs=2, space="PSUM") as ps_s,
            tc.tile_pool(name="ps_o", bufs=1, space="PSUM") as ps_o,
            tc.tile_pool(name="ps_p", bufs=2, space="PSUM") as ps_p,
        ):
            # ---- constants / weights (q-proj inputs first) ----
            w1t = const.tile([128, 4, HG_CH], BF16)
            b1c = const.tile([128, 2], F32)
            qts = big.tile([128, 4, NP], BF16)
            nc.sync.dma_start(out=w1t[:], in_=w1t_d[:])
            nc.sync.dma_start(out=b1c[:], in_=b1c_d[:])
            nc.sync.dma_start(out=qts[:, :, 0:512], in_=qt_d[:, :, 0:512])

            w2t = const.tile([128, 4, HG_CH], BF16)
            b2c = const.tile([128, 2], F32)
            nc.sync.dma_start(out=w2t[:], in_=w2t_d[:])
            nc.sync.dma_start(out=b2c[:], in_=b2c_d[:])

            w3t = const.tile([128, 4, 260], BF16)
            b3r = const.tile([1, 260], BF16)
            onesc = const.tile([128, 1], BF16)
            ones128 = const.tile([1, 128], BF16)
            ones64 = const.tile([1, 64], F32)
            const_dmas = [
                (w3t, w3t_d), (b3r, b3r_d),
                (onesc, ones_d), (ones128, ones128_d), (ones64, ones64_d),
            ]

            maskt = big.tile([128, NKC, NP], BF16)
            qhT = big.tile([128, 2, NP], BF16)   # [ch_in_pair, pair, query]
            khT = big.tile([128, 2, MNP], BF16)  # [ch_in_pair, pair, key]
            vh = big.tile([128, NKC, 260], BF16)  # [key_in_chunk, kc, 4*(64ch+1)]

            # first k/v chunk + tail of q before the bulk constants
            # staged k/v chunks, 4 kc (512 keys) per DMA; ring dicts keyed by nq
            k_stage = {}
            v_stage = {}

            def fetch_kv(nq):
                kts = kio.tile([128, 4, 512], BF16, tag="kts", name=f"kts{nq}")
                nc.sync.dma_start(out=kts[:], in_=kt_d[:, :, nq * 512:(nq + 1) * 512])
                k_stage[nq] = kts
                vts = vio.tile([128, 4, 512], BF16, tag="vts", name=f"vts{nq}")
                nc.sync.dma_start(out=vts[:], in_=vt_d[:, :, nq * 512:(nq + 1) * 512])
                v_stage[nq] = vts

            fetch_kv(0)
            nc.sync.dma_start(out=qts[:, :, 512:1024], in_=qt_d[:, :, 512:1024])

            # ---- qhT projection (nq-half as soon as its qts half lands) ----
            for nq in range(NP // 512):
                for pair in range(2):
                    pt = ps_p.tile([128, 512], F32, tag="pp")
                    for c in range(4):
                        nc.tensor.matmul(
                            pt[:],
                            lhsT=w1t[:, c, pair * 128:(pair + 1) * 128],
                            rhs=qts[:, c, nq * 512:(nq + 1) * 512],
                            start=(c == 0),
                            stop=(c == 3),
                        )
                    nc.scalar.activation(
                        qhT[:, pair, nq * 512:(nq + 1) * 512],
                        pt[:],
                        AF.Identity,
                        bias=b1c[:, pair:pair + 1],
                    )

            def proj_k_chunk(nq):
                kts = k_stage.pop(nq)
                for pair in range(2):
                    pt = ps_p.tile([128, 512], F32, tag="pp")
                    for c in range(4):
                        nc.tensor.matmul(
                            pt[:],
                            lhsT=w2t[:, c, pair * 128:(pair + 1) * 128],
                            rhs=kts[:, c, :],
                            start=(c == 0),
                            stop=(c == 3),
                        )
                    # eviction + bias on ScalarE: pass 0 is PE-bound, ScalarE
                    # has slack there, and this keeps the DVE queue short.
                    nc.scalar.activation(
                        khT[:, pair, nq * 512:(nq + 1) * 512],
                        pt[:],
                        AF.Identity,
                        bias=b2c[:, pair:pair + 1],
                    )

            def proj_v_chunk(kc):
                vts = v_stage[kc // 4]
                co = (kc % 4) * 128
                pt = ps_p.tile([128, 260], F32, tag="pp")
                for c in range(4):
                    nc.tensor.matmul(
                        pt[:],
                        lhsT=vts[:, c, co:co + 128],
                        rhs=w3t[:, c, :],
                        start=(c == 0),
                        stop=False,
                    )
                nc.tensor.matmul(
                    pt[:], lhsT=ones128[:], rhs=b3r[:], start=False, stop=True,
                )
                nc.vector.tensor_copy(out=vh[:, kc, :], in_=pt[:])
                if kc % 4 == 3:
                    del v_stage[kc // 4]

            def issue_av(qt, hp, kc, numerm, o_ps):
                for h in range(2):
                    ch = (2 * hp + h) * 65
                    nc.tensor.matmul(
                        o_ps[h][:],
                        lhsT=vh[:, kc, ch:ch + 65],
                        rhs=numerm[:, h, :],
                        start=(kc == 0),
                        stop=(kc == NKC - 1),
                    )

            def attn_tail_a(qt, hp, o_ps):
                # Copies only: they are the last readers of o_ps, so the next
                # pass's AV accumulators can start while the recip path runs.
                o_sb = fin.tile([128, 512], F32, tag="osb")
                dsbs = []
                for h in range(2):
                    dsb = work.tile([1, 512], F32, tag=f"dsb{h}")
                    nc.vector.tensor_copy(out=dsb[:], in_=o_ps[h][64:65, :])
                    nc.vector.tensor_copy(
                        out=o_sb[h * 64:(h + 1) * 64, :], in_=o_ps[h][0:64, :]
                    )
                    dsbs.append(dsb)
                return o_sb, dsbs

            def attn_tail_b(qt, hp, o_sb, dsbs):
                # Deferred into the next pass so the recip DMA round-trips and
                # the b_ps matmuls never stall the main PE stream.
                b_ps = ps_p.tile([128, 512], F32, tag="pp")
                for h in range(2):
                    drow = work.tile([128, 4], F32, tag=f"drow{h}")
                    nc.sync.dma_start(out=drow[:], in_=dsbs[h][:])
                    rrow = work.tile([128, 4], F32, tag=f"rrow{h}")
                    nc.vector.reciprocal(rrow[:], drow[:])
                    rec = work.tile([1, 512], F32, tag=f"rec{h}")
                    nc.sync.dma_start(out=rec[:], in_=rrow[:])
                    nc.tensor.matmul(
                        b_ps[h * 64:(h + 1) * 64, :],
                        lhsT=ones64[:],
                        rhs=rec[:],
                        start=True,
                        stop=True,
                    )
                outt = fin.tile([128, 512], F32, tag="outt")
                nc.vector.tensor_tensor(
                    out=outt[:], in0=o_sb[:], in1=b_ps[:], op=ALU.mult
                )
                nc.sync.dma_start(out=out_d[hp, :, qt, :], in_=outt[:])

            # ---- main pipeline: 4 passes over (qt, hp), proj in pass 0 ----
            def fetch_mask(nq, half):
                nc.sync.dma_start(
                    out=maskt[:, 4 * nq:4 * (nq + 1), half * 512:(half + 1) * 512],
                    in_=mask_d[:, 4 * nq:4 * (nq + 1), half * 512:(half + 1) * 512],
                )

            pend_tail = None
            for pi, (qt, hp) in enumerate(((0, 0), (0, 1), (1, 0), (1, 1))):
                o_ps = [
                    ps_o.tile([65, 512], F32, tag=f"o{h}", name=f"ops{qt}{hp}{h}")
                    for h in range(2)
                ]
                pend = []
                for kc in range(NKC):
                    nq = kc // 4
                    if pi == 0:
                        if kc == 0:
                            fetch_mask(0, 0)
                            fetch_kv(1)
                            for sb, dr in const_dmas:
                                nc.sync.dma_start(out=sb[:], in_=dr[:])
                            fetch_mask(1, 0)
                        if kc % 4 == 0:
                            if nq + 2 < 8:
                                fetch_kv(nq + 2)
                                fetch_mask(nq + 2, 0)
                            proj_k_chunk(nq)
                    elif pi == 1 and kc % 4 == 0:
                        fetch_mask(nq, 1)
                    # scores: row-tiled concurrent pair (K=64 each)
                    s = ps_s.tile([128, 2, 512], F32, tag="s")
                    for h in range(2):
                        nc.tensor.matmul(
                            s[:, h, :],
                            lhsT=khT[h * 64:(h + 1) * 64, hp, kc * 128:(kc + 1) * 128],
                            rhs=qhT[h * 64:(h + 1) * 64, hp, qt * 512:(qt + 1) * 512],
                            start=True,
                            stop=True,
                        )
                    # deferred previous-pass finisher: PE is rolling again now
                    if kc == 3 and pend_tail is not None:
                        attn_tail_b(*pend_tail)
                        pend_tail = None
                    # AV lags the score wave by 3 kc to absorb exp/mask jitter
                    if len(pend) == 3:
                        issue_av(qt, hp, *pend.pop(0), o_ps)
                    # v-proj after the wave: w3t/vt arrive later than w2t/kt,
                    # and vh[kc] is not needed until AV(kc) three waves on
                    if pi == 0:
                        proj_v_chunk(kc)
                    numer = work.tile([128, 2, 512], BF16, tag="numer")
                    nc.scalar.activation(numer[:], s[:], AF.Exp)
                    numerm = work.tile([128, 2, 512], BF16, tag="numerm", bufs=6)
                    nc.vector.tensor_tensor(
                        out=numerm[:],
                        in0=numer[:],
                        in1=maskt[:, kc:kc + 1, qt * 512:(qt + 1) * 512].to_broadcast(
                            (128, 2, 512)
                        ),
                        op=ALU.mult,
                    )
                    pend.append((kc, numerm))
                for p in pend:
                    issue_av(qt, hp, *p, o_ps)
                o_sb, dsbs = attn_tail_a(qt, hp, o_ps)
                pend_tail = (qt, hp, o_sb, dsbs)
            attn_tail_b(*pend_tail)

    _split_multi_waits(nc)
    return nc


def _prep_inputs(q, k, v, rns_indices, W1, b1, W2, b2, W3, b3):
    bf = ml_dtypes.bfloat16
    q = np.asarray(q, np.float32)
    k = np.asarray(k, np.float32)
    v = np.asarray(v, np.float32)
    idx = np.asarray(rns_indices)
    W1 = np.asarray(W1, np.float32)
    W2 = np.asarray(W2, np.float32)
    W3 = np.asarray(W3, np.float32)
    b1 = np.asarray(b1, np.float32)
    b2 = np.asarray(b2, np.float32)
    b3 = np.asarray(b3, np.float32)
    scale = 1.0 / np.sqrt(DH)

    def part3(x2d, n):  # (512, n) -> (128, 4, n)
        return np.ascontiguousarray(
            x2d.reshape(4, 128, n).transpose(1, 0, 2)
        ).astype(bf)

    def _aug_w3(W3h):  # (256, 512) -> (128, 4, 260) with zero cols at ones slots
        wt = np.zeros((DIM, 260), np.float32)
        for h in range(4):
            wt[:, h * 65:h * 65 + 64] = W3h[h * 64:(h + 1) * 64, :].T
        return part3(wt, 260)

    def _aug_b3(b3h):  # (256,) -> (1, 260) with 1.0 at ones slots
        br = np.zeros((1, 260), np.float32)
        for h in range(4):
            br[0, h * 65:h * 65 + 64] = b3h[h * 64:(h + 1) * 64]
            br[0, h * 65 + 64] = 1.0
        return br.astype(bf)

    masks = []
    for b in range(BS):
        m = np.zeros((NP, MNP), np.float32)
        m[np.arange(NP)[:, None], idx[b]] = 1.0
        mt = m.T.reshape(NKC, 128, NP).transpose(1, 0, 2)
        masks.append(np.ascontiguousarray(mt).astype(bf))

    qkv_t = []
    for b in range(BS):
        qkv_t.append(
            (
                part3(q[:, b, :].T, NP),
                part3(k[:, b, :].T, MNP),
                part3(v[:, b, :].T, MNP),
            )
        )

    in_maps = []
    for core in range(N_CORES):
        b, hg = core // 2, core % 2
        sl = slice(hg * HG_CH, (hg + 1) * HG_CH)
        qtb, ktb, vtb = qkv_t[b]
        im = {
            "qt": qtb,
            "kt": ktb,
            "vt": vtb,
            "w1t": part3(W1[sl, :].T * scale, HG_CH),
            "w2t": part3(W2[sl, :].T, HG_CH),
            "w3t": _aug_w3(W3[sl, :]),
            "b1c": np.ascontiguousarray(
                (b1[sl] * scale).reshape(2, 128).T
            ).astype(np.float32),
            "b2c": np.ascontiguousarray(b2[sl].reshape(2, 128).T).astype(np.float32),
            "b3r": _aug_b3(b3[sl]),
            "maskt": masks[b],
            "onesc": np.ones((128, 1), bf),
            "ones128": np.ones((1, 128), bf),
            "ones64": np.ones((1, 64), np.float32),
        }
        in_maps.append(im)
    return in_maps


def kernel(q, k, v, rns_indices, W1, b1, W2, b2, W3, b3):
    nc = _build_nc()
    in_maps = _prep_inputs(q, k, v, rns_indices, W1, b1, W2, b2, W3, b3)
    res = run_bass_kernel_spmd(
        nc,
        in_maps,
        core_ids=list(range(N_CORES)),
        trace=run_opts["trace"],
        **run_opts["trace_kwargs"],
    )
    _last_results["res"] = res

    out = np.empty((NP, BS, DIM), np.float32)
    for core in range(N_CORES):
        b, hg = core // 2, core % 2
        r = np.asarray(res.results[core]["outt"], np.float32)  # (2,128,2,512)
        arr = r.transpose(2, 3, 0, 1).reshape(NP, HG_CH)
        out[:, b, hg * HG_CH:(hg + 1) * HG_CH] = arr
    return out


# revision 22
# speedup vs baseline: 1.2049x; 1.0026x over previous
"""Sparse cross-attention kernel for Trainium2 (8 NeuronCores, SPMD).

Problem: nn_CrossAttn (NP=1024 queries, MNP=4096 keys, BS=4, DIM=512,
NH=8 heads, dh=64, TOPK=32 sparse mask shared across heads).

Sharding: core = (batch b, head-group hg).  4 batches x 2 head-groups of 4
heads each.  Each core computes its batch's attention for its 4 heads and
writes a (256 ch, 1024 q) transposed output block; the host reassembles.

v2 layout: ScalarE exp of the dense masked scores is the hard floor
(~1.1us per (qt,hp,kc) unit, 128 units).  The kernel is a single software
pipeline that keeps ScalarE saturated from ~5us on:
  - JIT chunk DMAs (kt/vt/mask per kc; mask split by qt-half over passes 1-2)
  - projections interleaved across all 32 kc of pass 1
  - scores as a row-tiled concurrent pair (2 heads, K=64 each)
  - AV (den folded as 65th channel) issued one kc behind the score wave
  - tails release AV PSUM accumulators early (copies first, recip after)
"""

import numpy as np
import ml_dtypes

import concourse.bass as bass
import concourse.mybir as mybir
import concourse.tile as tile
from concourse.bass_utils import run_bass_kernel_spmd

BF16 = mybir.dt.bfloat16
F32 = mybir.dt.float32
AF = mybir.ActivationFunctionType
ALU = mybir.AluOpType

NH = 8
DIM = 512
NP = 1024
MNP = 4096
BS = 4
DH = 64
N_CORES = 8
HG_CH = 256          # channels per head-group (4 heads x 64)
NKC = MNP // 128     # 32 key chunks
NQT = NP // 512      # 2 query tiles

# options test.py can flip
run_opts = {"trace": False, "trace_kwargs": {}}
_last_results = {}


def _split_multi_waits(nc):
    """This container's walrus encodes only ONE sync-wait per TPB instruction
    (newer Tile emits several).  Split extras onto preceding NOPs."""
    eng_ok = {
        mybir.EngineType.PE,
        mybir.EngineType.Activation,
        mybir.EngineType.DVE,
        mybir.EngineType.Pool,
        mybir.EngineType.SP,
    }
    for fn in nc.m.functions:
        for blk in fn.blocks:
            insts = blk.instructions
            out = []
            changed = False
            for inst in insts:
                si = inst.sync_info
                if (
                    si is not None
                    and si.on_wait
                    and len(si.on_wait) > 1
                    and inst.engine in eng_ok
                ):
                    waits = list(si.on_wait)
                    for j, w in enumerate(waits[:-1]):
                        out.append(
                            mybir.InstNoOp(
                                name=f"{inst.name}-w{j}",
                                engine=inst.engine,
                                ins=[],
                                outs=[],
                                sync_info=mybir.SyncInfo(on_wait=[w], on_update=[]),
                            )
                        )
                    inst.sync_info = mybir.SyncInfo(
                        on_wait=[waits[-1]], on_update=list(si.on_update)
                    )
                    changed = True
                out.append(inst)
            if changed:
                blk.instructions = out


def _build_nc() -> bass.Bass:
    nc = bass.Bass()

    qt_d = nc.dram_tensor("qt", [128, 4, NP], BF16, kind="ExternalInput")
    kvt_d = nc.dram_tensor("kvt", [128, 2, 4, MNP], BF16, kind="ExternalInput")
    w1t_d = nc.dram_tensor("w1t", [128, 4, HG_CH], BF16, kind="ExternalInput")
    w2t_d = nc.dram_tensor("w2t", [128, 4, HG_CH], BF16, kind="ExternalInput")
    w3t_d = nc.dram_tensor("w3t", [128, 4, 260], BF16, kind="ExternalInput")
    b1c_d = nc.dram_tensor("b1c", [128, 2], F32, kind="ExternalInput")
    b2c_d = nc.dram_tensor("b2c", [128, 2], F32, kind="ExternalInput")
    b3r_d = nc.dram_tensor("b3r", [1, 260], BF16, kind="ExternalInput")
    mask_d = nc.dram_tensor("maskt", [128, NKC, NP], BF16, kind="ExternalInput")
    ones_d = nc.dram_tensor("onesc", [128, 1], BF16, kind="ExternalInput")
    ones128_d = nc.dram_tensor("ones128", [1, 128], BF16, kind="ExternalInput")
    ones64_d = nc.dram_tensor("ones64", [1, 64], F32, kind="ExternalInput")
    out_d = nc.dram_tensor("outt", [2, 128, NQT, 512], F32, kind="ExternalOutput")

    with tile.TileContext(nc) as tc:
        with (
            tc.tile_pool(name="const", bufs=1) as const,
            tc.tile_pool(name="big", bufs=1) as big,
            tc.tile_pool(name="kio", bufs=3) as kio,
            tc.tile_pool(name="work", bufs=4) as work,
            tc.tile_pool(name="fin", bufs=2) as fin,
            tc.tile_pool(name="ps_s", bufs=2, space="PSUM") as ps_s,
            tc.tile_pool(name="ps_o", bufs=1, space="PSUM") as ps_o,
            tc.tile_pool(name="ps_p", bufs=2, space="PSUM") as ps_p,
        ):
            # ---- constants / weights (q-proj inputs first) ----
            w1t = const.tile([128, 4, HG_CH], BF16)
            b1c = const.tile([128, 2], F32)
            qts = big.tile([128, 4, NP], BF16)
            nc.sync.dma_start(out=w1t[:], in_=w1t_d[:])
            nc.sync.dma_start(out=b1c[:], in_=b1c_d[:])
            nc.sync.dma_start(out=qts[:, :, 0:512], in_=qt_d[:, :, 0:512])

            w2t = const.tile([128, 4, HG_CH], BF16)
            b2c = const.tile([128, 2], F32)
            nc.sync.dma_start(out=w2t[:], in_=w2t_d[:])
            nc.sync.dma_start(out=b2c[:], in_=b2c_d[:])

            w3t = const.tile([128, 4, 260], BF16)
            b3r = const.tile([1, 260], BF16)
            onesc = const.tile([128, 1], BF16)
            ones128 = const.tile([1, 128], BF16)
            ones64 = const.tile([1, 64], F32)
            const_dmas = [
                (w3t, w3t_d), (b3r, b3r_d),
                (onesc, ones_d), (ones128, ones128_d), (ones64, ones64_d),
            ]

            maskt = big.tile([128, NKC, NP], BF16)
            qhT = big.tile([128, 2, NP], BF16)   # [ch_in_pair, pair, query]
            khT = big.tile([128, 2, MNP], BF16)  # [ch_in_pair, pair, key]
            vh = big.tile([128, NKC, 260], BF16)  # [key_in_chunk, kc, 4*(64ch+1)]

            # staged k/v chunks: one DMA brings 4 kc (512 keys) of both k and v
            kv_stage = {}

            def fetch_kv(nq):
                kvs = kio.tile([128, 2, 4, 512], BF16, tag="kvs", name=f"kvs{nq}")
                nc.sync.dma_start(
                    out=kvs[:], in_=kvt_d[:, :, :, nq * 512:(nq + 1) * 512]
                )
                kv_stage[nq] = kvs

            fetch_kv(0)
            nc.sync.dma_start(out=qts[:, :, 512:1024], in_=qt_d[:, :, 512:1024])

            # ---- qhT projection (nq-half as soon as its qts half lands) ----
            for nq in range(NP // 512):
                for pair in range(2):
                    pt = ps_p.tile([128, 512], F32, tag="pp")
                    for c in range(4):
                        nc.tensor.matmul(
                            pt[:],
                            lhsT=w1t[:, c, pair * 128:(pair + 1) * 128],
                            rhs=qts[:, c, nq * 512:(nq + 1) * 512],
                            start=(c == 0),
                            stop=(c == 3),
                        )
                    nc.scalar.activation(
                        qhT[:, pair, nq * 512:(nq + 1) * 512],
                        pt[:],
                        AF.Identity,
                        bias=b1c[:, pair:pair + 1],
                    )

            def proj_k_chunk(nq):
                kvs = kv_stage[nq]
                for pair in range(2):
                    pt = ps_p.tile([128, 512], F32, tag="pp")
                    for c in range(4):
                        nc.tensor.matmul(
                            pt[:],
                            lhsT=w2t[:, c, pair * 128:(pair + 1) * 128],
                            rhs=kvs[:, 0, c, :],
                            start=(c == 0),
                            stop=(c == 3),
                        )
                    # eviction + bias on ScalarE: pass 0 is PE-bound, ScalarE
                    # has slack there, and this keeps the DVE queue short.
                    nc.scalar.activation(
                        khT[:, pair, nq * 512:(nq + 1) * 512],
                        pt[:],
                        AF.Identity,
                        bias=b2c[:, pair:pair + 1],
                    )

            def proj_v_chunk(kc):
                kvs = kv_stage[kc // 4]
                co = (kc % 4) * 128
                pt = ps_p.tile([128, 260], F32, tag="pp")
                for c in range(4):
                    nc.tensor.matmul(
                        pt[:],
                        lhsT=kvs[:, 1, c, co:co + 128],
                        rhs=w3t[:, c, :],
                        start=(c == 0),
                        stop=False,
                    )
                nc.tensor.matmul(
                    pt[:], lhsT=ones128[:], rhs=b3r[:], start=False, stop=True,
                )
                nc.vector.tensor_copy(out=vh[:, kc, :], in_=pt[:])
                if kc % 4 == 3:
                    del kv_stage[kc // 4]

            def issue_av(qt, hp, kc, numerm, o_ps):
                for h in range(2):
                    ch = (2 * hp + h) * 65
                    nc.tensor.matmul(
                        o_ps[h][:],
                        lhsT=vh[:, kc, ch:ch + 65],
                        rhs=numerm[:, h, :],
                        start=(kc == 0),
                        stop=(kc == NKC - 1),
                    )

            def attn_tail_a(qt, hp, o_ps):
                # Copies only: they are the last readers of o_ps, so the next
                # pass's AV accumulators can start while the recip path runs.
                o_sb = fin.tile([128, 512], F32, tag="osb")
                dsbs = []
                for h in range(2):
                    dsb = work.tile([1, 512], F32, tag=f"dsb{h}")
                    nc.vector.tensor_copy(out=dsb[:], in_=o_ps[h][64:65, :])
                    nc.vector.tensor_copy(
                        out=o_sb[h * 64:(h + 1) * 64, :], in_=o_ps[h][0:64, :]
                    )
                    dsbs.append(dsb)
                return o_sb, dsbs

            def attn_tail_b(qt, hp, o_sb, dsbs):
                # Deferred into the next pass so the recip DMA round-trips and
                # the b_ps matmuls never stall the main PE stream.  Both heads'
                # chains run stage-parallel to halve the latency.
                b_ps = ps_p.tile([128, 512], F32, tag="pp")
                drows, rrows, recs = [], [], []
                for h in range(2):
                    drow = work.tile([128, 4], F32, tag=f"drow{h}")
                    nc.sync.dma_start(out=drow[:], in_=dsbs[h][:])
                    drows.append(drow)
                for h in range(2):
                    rrow = work.tile([128, 4], F32, tag=f"rrow{h}")
                    nc.vector.reciprocal(rrow[:], drows[h][:])
                    rrows.append(rrow)
                for h in range(2):
                    rec = work.tile([1, 512], F32, tag=f"rec{h}")
                    nc.sync.dma_start(out=rec[:], in_=rrows[h][:])
                    recs.append(rec)
                for h in range(2):
                    nc.tensor.matmul(
                        b_ps[h * 64:(h + 1) * 64, :],
                        lhsT=ones64[:],
                        rhs=recs[h][:],
                        start=True,
                        stop=True,
                    )
                outt = fin.tile([128, 512], F32, tag="outt")
                nc.vector.tensor_tensor(
                    out=outt[:], in0=o_sb[:], in1=b_ps[:], op=ALU.mult
                )
                nc.sync.dma_start(out=out_d[hp, :, qt, :], in_=outt[:])

            # ---- main pipeline: 4 passes over (qt, hp), proj in pass 0 ----
            def fetch_mask(nq, half):
                nc.sync.dma_start(
                    out=maskt[:, 4 * nq:4 * (nq + 1), half * 512:(half + 1) * 512],
                    in_=mask_d[:, 4 * nq:4 * (nq + 1), half * 512:(half + 1) * 512],
                )

            pend_tail = None
            for pi, (qt, hp) in enumerate(((0, 0), (0, 1), (1, 0), (1, 1))):
                o_ps = [
                    ps_o.tile([65, 512], F32, tag=f"o{h}", name=f"ops{qt}{hp}{h}")
                    for h in range(2)
                ]
                pend = []
                for kc in range(NKC):
                    nq = kc // 4
                    if pi == 0:
                        if kc == 0:
                            fetch_mask(0, 0)
                            fetch_kv(1)
                            for sb, dr in const_dmas:
                                nc.sync.dma_start(out=sb[:], in_=dr[:])
                            fetch_mask(1, 0)
                        if kc % 4 == 0:
                            if nq + 2 < 8:
                                fetch_kv(nq + 2)
                                fetch_mask(nq + 2, 0)
                            proj_k_chunk(nq)
                    elif pi == 1 and kc % 4 == 0:
                        fetch_mask(nq, 1)
                    # scores: row-tiled concurrent pair (K=64 each)
                    s = ps_s.tile([128, 2, 512], F32, tag="s")
                    for h in range(2):
                        nc.tensor.matmul(
                            s[:, h, :],
                            lhsT=khT[h * 64:(h + 1) * 64, hp, kc * 128:(kc + 1) * 128],
                            rhs=qhT[h * 64:(h + 1) * 64, hp, qt * 512:(qt + 1) * 512],
                            start=True,
                            stop=True,
                        )
                    # deferred previous-pass finisher: PE is rolling again now
                    # (kc 8: the recip DMA round-trips take ~6us to land)
                    if kc == 8 and pend_tail is not None:
                        attn_tail_b(*pend_tail)
                        pend_tail = None
                    # AV lags the score wave by 3 kc to absorb exp/mask jitter
                    if len(pend) == 3:
                        issue_av(qt, hp, *pend.pop(0), o_ps)
                    # v-proj after the wave: w3t/vt arrive later than w2t/kt,
                    # and vh[kc] is not needed until AV(kc) three waves on
                    if pi == 0:
                        proj_v_chunk(kc)
                    numer = work.tile([128, 2, 512], BF16, tag="numer")
                    nc.scalar.activation(numer[:], s[:], AF.Exp)
                    numerm = work.tile([128, 2, 512], BF16, tag="numerm", bufs=6)
                    nc.vector.tensor_tensor(
                        out=numerm[:],
                        in0=numer[:],
                        in1=maskt[:, kc:kc + 1, qt * 512:(qt + 1) * 512].to_broadcast(
                            (128, 2, 512)
                        ),
                        op=ALU.mult,
                    )
                    pend.append((kc, numerm))
                for p in pend:
                    issue_av(qt, hp, *p, o_ps)
                o_sb, dsbs = attn_tail_a(qt, hp, o_ps)
                pend_tail = (qt, hp, o_sb, dsbs)
            attn_tail_b(*pend_tail)

    _split_multi_waits(nc)
    return nc


def _prep_inputs(q, k, v, rns_indices, W1, b1, W2, b2, W3, b3):
    bf = ml_dtypes.bfloat16
    q = np.asarray(q, np.float32)
    k = np.asarray(k, np.float32)
    v = np.asarray(v, np.float32)
    idx = np.asarray(rns_indices)
    W1 = np.asarray(W1, np.float32)
    W2 = np.asarray(W2, np.float32)
    W3 = np.asarray(W3, np.float32)
    b1 = np.asarray(b1, np.float32)
    b2 = np.asarray(b2, np.float32)
    b3 = np.asarray(b3, np.float32)
    scale = 1.0 / np.sqrt(DH)

    def part3(x2d, n):  # (512, n) -> (128, 4, n)
        return np.ascontiguousarray(
            x2d.reshape(4, 128, n).transpose(1, 0, 2)
        ).astype(bf)

    def _aug_w3(W3h):  # (256, 512) -> (128, 4, 260) with zero cols at ones slots
        wt = np.zeros((DIM, 260), np.float32)
        for h in range(4):
            wt[:, h * 65:h * 65 + 64] = W3h[h * 64:(h + 1) * 64, :].T
        return part3(wt, 260)

    def _aug_b3(b3h):  # (256,) -> (1, 260) with 1.0 at ones slots
        br = np.zeros((1, 260), np.float32)
        for h in range(4):
            br[0, h * 65:h * 65 + 64] = b3h[h * 64:(h + 1) * 64]
            br[0, h * 65 + 64] = 1.0
        return br.astype(bf)

    masks = []
    for b in range(BS):
        m = np.zeros((NP, MNP), np.float32)
        m[np.arange(NP)[:, None], idx[b]] = 1.0
        mt = m.T.reshape(NKC, 128, NP).transpose(1, 0, 2)
        masks.append(np.ascontiguousarray(mt).astype(bf))

    qkv_t = []
    for b in range(BS):
        ktb = part3(k[:, b, :].T, MNP)
        vtb = part3(v[:, b, :].T, MNP)
        qkv_t.append(
            (
                part3(q[:, b, :].T, NP),
                np.ascontiguousarray(np.stack([ktb, vtb], axis=1)),
            )
        )

    in_maps = []
    for core in range(N_CORES):
        b, hg = core // 2, core % 2
        sl = slice(hg * HG_CH, (hg + 1) * HG_CH)
        qtb, kvtb = qkv_t[b]
        im = {
            "qt": qtb,
            "kvt": kvtb,
            "w1t": part3(W1[sl, :].T * scale, HG_CH),
            "w2t": part3(W2[sl, :].T, HG_CH),
            "w3t": _aug_w3(W3[sl, :]),
            "b1c": np.ascontiguousarray(
                (b1[sl] * scale).reshape(2, 128).T
            ).astype(np.float32),
            "b2c": np.ascontiguousarray(b2[sl].reshape(2, 128).T).astype(np.float32),
            "b3r": _aug_b3(b3[sl]),
            "maskt": masks[b],
            "onesc": np.ones((128, 1), bf),
            "ones128": np.ones((1, 128), bf),
            "ones64": np.ones((1, 64), np.float32),
        }
        in_maps.append(im)
    return in_maps


def kernel(q, k, v, rns_indices, W1, b1, W2, b2, W3, b3):
    nc = _build_nc()
    in_maps = _prep_inputs(q, k, v, rns_indices, W1, b1, W2, b2, W3, b3)
    res = run_bass_kernel_spmd(
        nc,
        in_maps,
        core_ids=list(range(N_CORES)),
        trace=run_opts["trace"],
        **run_opts["trace_kwargs"],
    )
    _last_results["res"] = res

    out = np.empty((NP, BS, DIM), np.float32)
    for core in range(N_CORES):
        b, hg = core // 2, core % 2
        r = np.asarray(res.results[core]["outt"], np.float32)  # (2,128,2,512)
        arr = r.transpose(2, 3, 0, 1).reshape(NP, HG_CH)
        out[:, b, hg * HG_CH:(hg + 1) * HG_CH] = arr
    return out
